# revision 28
# baseline (speedup 1.0000x reference)
"""Trainium2 Bass kernel for a 3-layer binarized CNN (tunnel-optimized).

Network (reference):
    x  : [32, 3, 512, 512] fp32
    l1 : clip(conv(x, sign(w1)))            -> [32,16,510,510]
    l2 : clip(conv(sign(l1), sign(w2)))     -> [32,23,508,508]
    l3 : clip(conv(sign(l2), sign(w3)))     -> [32,2,506,506]
    out: l3.reshape(32, -1)

The axon tunnel moves ~100 MB/s up / ~50 MB/s down with substantial
per-call overhead, so end-to-end wall time is dominated by host work and
bytes shipped, not device cycles.  Strategy:

  * Layer 1 only matters through the SIGN of its fp32 output.  It is
    computed on the host (one 16x27 sgemm per image, 512-wide junk-col
    im2col + uint64 sign-bit gather) and shipped as bit-packed signs:
    17 MB instead of the 100 MB fp32 input.
  * The device unpacks bits to +-1 fp8 activations (DVE shift/and ops),
    then runs layers 2+3 exactly as integer-exact fp8 DoubleRow
    Toeplitz matmuls (data parallel over images).
  * The output (values in {-1,0,1}) is packed 5 columns/byte as
    balanced-ternary digits: p = s0+3*s1+9*s2+27*s3+81*s4 in int8,
    cutting the returned tensor 5x.  The host decodes with a 256x5 LUT.
  * A single jax.jit executable is traced once and reused every round;
    the Toeplitz weights ride up once as device-resident arrays and the
    donated output zero-buffers are created on-device, so per-round
    tunnel traffic is just packed signs up + ternary output down.
  * Rounds pipeline through a 1-worker executor: round k+1's host encode
    overlaps round k's transfer + execution.
"""

import numpy as np

import concourse.bacc as bacc
import concourse.mybir as mybir
import concourse.tile as tile
from concourse import bass2jax

F32 = mybir.dt.float32
F16 = mybir.dt.float16
F8 = mybir.dt.float8e4
U8 = mybir.dt.uint8
I8 = mybir.dt.int8
ALU = mybir.AluOpType
DR = mybir.MatmulPerfMode.DoubleRow
SIGN = mybir.ActivationFunctionType.Sign

N_CORES = 8
AL2, AO2 = 7, 5      # L2: rows window / rows out per block
AL3, AO3 = 32, 30    # L3
C2, C3 = 16, 23
O2, O3 = 23, 2


def _toeplitz_weights(w2, w3):
    """Build the stationary Toeplitz matrices (host side)."""
    s2 = np.sign(w2).astype(np.float32)  # [23,16,3,3]
    s3 = np.sign(w3).astype(np.float32)  # [2,23,3,3]

    # T2[(al*8 + cp), dx, codd, (aol*23 + PI[o])] fp8 DoubleRow pairs, M pad
    # 128.  PI permutes the 23 output channels into the (c4, cc, t) order of
    # the s2 DRAM layout so layer 3 can load its rhs with a single DMA:
    # o = cc*8 + c4*2 + t  is stored at  c' = c4*6 + cc*2 + t.
    PI = [((o % 8) // 2) * 6 + (o // 8) * 2 + (o % 2) for o in range(O2)]
    t2 = np.zeros((56, 3, 2, 128), np.float32)
    for al in range(AL2):
        for aol in range(AO2):
            dy = al - aol
            if 0 <= dy <= 2:
                for c in range(C2):
                    for o in range(O2):
                        t2[al * 8 + c // 2, :, c % 2, aol * 23 + PI[o]] = s2[o, c, dy, :]
    # T3[(al*4 + cp), cc, dx, codd, (o*30 + aol)] fp8 DoubleRow, M pad 64.
    # M index is o-major so the output lands channel-major in DRAM.
    t3 = np.zeros((128, 3, 3, 2, 64), np.float32)
    for al in range(AL3):
        for aol in range(AO3):
            dy = al - aol
            if 0 <= dy <= 2:
                for cc in range(3):
                    for cl in range(8):
                        c = cc * 8 + cl
                        if c < C3:
                            for o in range(O3):
                                t3[al * 4 + cl // 2, cc, :, cl % 2, o * 30 + aol] = (
                                    s3[o, c, dy, :]
                                )
    import ml_dtypes

    return (
        t2.reshape(56, 3 * 256).astype(ml_dtypes.float8_e4m3).view(np.uint8),
        t3.reshape(128, 9 * 128).astype(ml_dtypes.float8_e4m3).view(np.uint8),
    )


PIMG_BYTES = 512 * 16 * 64         # packed l1 signs per image (512 rows, 2 pad)


def _build_program(n_img, A, B):
    """Emit the per-core SPMD Bass program (unpack + layers 2,3)."""
    n1, n2, n3 = B - 2, B - 4, B - 6          # 510, 508, 506
    nblk2 = -(-(A - 4) // AO2)                 # 102
    nblk3 = -(-(A - 6) // AO3)                 # 17
    a1 = AL2 + AO2 * (nblk2 - 1)               # 512 s1 rows (incl zero tail)
    s2a = max(AO2 * nblk2, AO3 * (nblk3 - 1) + AL3)  # 512
    npk = (n3 + 4) // 5                        # 102 packed output bytes (base-3)

    nc = bacc.Bacc("TRN2", target_bir_lowering=False, debug=False)

    # pk layout per image: [16 ch, 512 rows, 64 B] (channel-major: the host
    # packer writes gemm output blocks without a transpose; the unpack DMA
    # below does the (row%16, ch-pair) partition interleave instead).
    pk_t = nc.dram_tensor("pk", [n_img * PIMG_BYTES], U8, kind="ExternalInput")
    s1p = pk_t.ap().rearrange(
        "(i c2 t g r16 b) -> i r16 c2 g t b",
        i=n_img, c2=8, t=2, g=a1 // 16, r16=16, b=64,
    )
    t2t = nc.dram_tensor("t2w", [56, 3 * 256], U8, kind="ExternalInput")
    t3t = nc.dram_tensor("t3w", [128, 9 * 128], U8, kind="ExternalInput")
    t2w = t2t.ap().bitcast(F8)
    t3w = t3t.ap().bitcast(F8)
    outp = nc.dram_tensor(
        "outp", [n_img, 2, AO3 * nblk3, npk], I8, kind="ExternalOutput"
    )
    # one spare junk row (index 512): the last L2 slab's second half-load
    # reads 8 rows starting at 505; row 512 is loaded but never consumed.
    s1d = [
        nc.dram_tensor(f"s1_{i}", [a1 + 1, 16, n1], F8, kind="Internal")
        for i in range(n_img)
    ]
    s2d = [
        nc.dram_tensor(f"s2_{i}", [s2a, 24, n2], F8, kind="Internal")
        for i in range(n_img)
    ]

    ngrp = a1 // 16                            # 32 packed row-groups
    nslab = (AO2 * nblk2) // 10                # 51 L2 double-block slabs

    with tile.TileContext(nc) as tc:
        with (
            tc.tile_pool(name="const", bufs=1) as cpool,
            tc.tile_pool(name="unp", bufs=2) as pu,
            tc.tile_pool(name="l2", bufs=4) as p2,
            tc.tile_pool(name="l3", bufs=4) as p3,
            tc.tile_pool(name="ps2", bufs=2, space="PSUM") as ps2p,
            tc.tile_pool(name="ps3", bufs=2, space="PSUM") as ps3p,
        ):
            # t2 stationary duplicated at partition bases 0 and 64: matmul
            # requires lhsT.base_partition == rhs.base_partition, and the
            # second block of each L2 slab reads rhs partitions 64..119.
            t2sb = cpool.tile([120, 3 * 256], F8)
            t3sb = cpool.tile([128, 9 * 128], F8)
            ztile = cpool.tile([128, B], F8)
            nc.sync.dma_start(t2sb[0:56, :], t2w)
            nc.sync.dma_start(t2sb[64:120, :], t2w)
            nc.sync.dma_start(t3sb[:], t3w)
            nc.vector.memset(ztile[:], 0.0)

            for img in range(n_img):
                s1, s2 = s1d[img].ap(), s2d[img].ap()
                # ---- zero pads of s2: channel-23 plane + tail rows ----
                for r in range(0, s2a, 128):
                    cnt = min(128, s2a - r)
                    nc.sync.dma_start(s2[r : r + cnt, 23, :], ztile[:cnt, :n2])
                for a in range(AO2 * nblk2, s2a):
                    nc.sync.dma_start(s2[a, :, :], ztile[:24, :n2])

                # ------- unpack host L1 sign bits -> s1, whole image -------
                # partitions (row%16, ch-pair); free dims (group, bytes)
                pk = pu.tile([128, 128 * ngrp], U8, tag="pk")
                pkv = pk[:].rearrange(
                    "(r16 c2) (g t b) -> c2 t r16 g b", r16=16, g=ngrp, t=2
                )
                for c2 in range(8):
                    for t in range(2):
                        nc.sync.dma_start(
                            pkv[c2, t], s1p[img][:, c2, :, t, :]
                        )
                upk = pu.tile([128, 1024 * ngrp], F8, tag="upk")
                upkv = upk[:].rearrange("p (q f) -> p q f", f=8)
                for k in range(8):
                    tb = pu.tile([128, 128 * ngrp], U8, tag="tb")
                    nc.vector.tensor_scalar(
                        tb[:], pk[:], int(7 - k), int(1),
                        op0=ALU.logical_shift_right, op1=ALU.bitwise_and,
                    )
                    # host packs raw sign bits (1 = negative): sign = 1 - 2b
                    nc.vector.tensor_scalar(
                        upkv[:, :, k], tb[:], -2.0, 1.0,
                        op0=ALU.mult, op1=ALU.add,
                    )
                for tt in range(2):
                    nc.sync.dma_start(
                        s1[0:a1].rearrange(
                            "(g r) (c t) n -> (r c) t g n", g=ngrp, t=2
                        )[:, tt],
                        upk[:].rearrange("p (g t n) -> p t g n", g=ngrp, t=2)[
                            :, tt, :, 0:n1
                        ],
                    )

                # --- layer 2: 10-row slabs as two 8-row half-loads (bases
                # 0/64), 2 blocks x 3 dx, fp8 DR, 2 PSUM banks, 1 Sign ---
                for s in range(nslab):
                    r0 = 10 * s
                    slab = p2.tile([128, 2 * 512], F8, tag="slab")
                    sv = slab[:].rearrange("k (t h) -> k t h", t=2)
                    nc.sync.dma_start(sv[0:64, :, 0:n1], s1[r0 : r0 + 8, :, :])
                    nc.sync.dma_start(
                        sv[64:128, :, 0:n1], s1[r0 + 5 : r0 + 13, :, :]
                    )
                    ps = ps2p.tile([115, 1024], F32, tag="ps2")
                    psv = ps[:].rearrange("p (g n) -> p g n", g=2)
                    for g in range(2):
                        for dx in range(3):
                            nc.tensor.matmul(
                                psv[:, g, 0:n2],
                                t2sb[64 * g : 64 * g + 56, :].rearrange(
                                    "k (x t m) -> k x t m", x=3, t=2
                                )[:, dx, :, 0:115],
                                sv[64 * g : 64 * g + 56, :, dx : dx + n2],
                                start=(dx == 0),
                                stop=(dx == 2),
                                perf_mode=DR,
                            )
                    sg2 = p2.tile([115, 2 * n2], F8, tag="sg2")
                    sg2v = sg2[:].rearrange("p (g n) -> p g n", g=2)
                    nc.scalar.activation(sg2v[:, :, :], psv[:, :, 0:n2], SIGN)
                    for g in range(2):
                        nc.sync.dma_start(
                            s2[r0 + 5 * g : r0 + 5 * g + 5, 0:23, :],
                            sg2v[:, g, :],
                        )

                # -------- layer 3 (fp8 DoubleRow) + 2-bit output pack --------
                for bb in range(nblk3):
                    rt = p3.tile([128, 3 * 2 * 512], F8, tag="rhs3")
                    rtv = rt[:].rearrange("k (b t h) -> k b t h", b=3, t=2)
                    nc.sync.dma_start(
                        rtv[:, :, :, 0:n2],
                        s2[30 * bb : 30 * bb + 32].rearrange(
                            "r (A b t) n -> (r A) b t n", A=4, t=2
                        ),
                    )
                    ps = ps3p.tile([60, n3], F32, tag="ps3")
                    for cc in range(3):
                        for dx in range(3):
                            nc.tensor.matmul(
                                ps[:],
                                t3sb[
                                    :, 128 * (cc * 3 + dx) : 128 * (cc * 3 + dx) + 128
                                ].rearrange("k (t m) -> k t m", t=2)[:, :, 0:60],
                                rtv[:, cc, :, dx : dx + n3],
                                start=(cc == 0 and dx == 0),
                                stop=(cc == 2 and dx == 2),
                                perf_mode=DR,
                            )
                    sg = p3.tile([60, 5 * npk], F16, tag="sg")
                    nc.vector.memset(sg[:, n3 : 5 * npk], 0.0)
                    nc.scalar.activation(sg[:, 0:n3], ps[:], SIGN)
                    sgv = sg[:].rearrange("p (n f) -> p n f", f=5)
                    pa = p3.tile([60, npk], F16, tag="pa")
                    nc.vector.scalar_tensor_tensor(
                        pa[:], sgv[:, :, 1], 3.0, sgv[:, :, 0],
                        op0=ALU.mult, op1=ALU.add,
                    )
                    pb = p3.tile([60, npk], F16, tag="pb")
                    nc.vector.scalar_tensor_tensor(
                        pb[:], sgv[:, :, 2], 9.0, pa[:],
                        op0=ALU.mult, op1=ALU.add,
                    )
                    pc = p3.tile([60, npk], F16, tag="pc")
                    nc.vector.scalar_tensor_tensor(
                        pc[:], sgv[:, :, 3], 27.0, pb[:],
                        op0=ALU.mult, op1=ALU.add,
                    )
                    po = p3.tile([60, npk], I8, tag="po")
                    nc.vector.scalar_tensor_tensor(
                        po[:], sgv[:, :, 4], 81.0, pc[:],
                        op0=ALU.mult, op1=ALU.add,
                    )
                    nc.sync.dma_start(
                        outp.ap()[img, :, 30 * bb : 30 * bb + 30, :], po[:]
                    )

    nc.compile()
    return nc


def _make_runner(nc):
    """Trace one reusable jax.jit for the SPMD program.

    Inputs ride as globally-concatenated arrays sharded over the 8 cores.
    The donated output zero-buffers are produced on-device by a second
    tiny jit, so they never cross the tunnel.
    Returns (call, put) where call(pk_np_or_jax, *resident) -> [np outs]
    and put(np_global) uploads a resident (replicated-by-tiling) input.
    """
    import jax
    import jax.numpy as jnp
    from jax.sharding import Mesh, PartitionSpec, NamedSharding
    from jax.experimental.shard_map import shard_map

    bass2jax.install_neuronx_cc_hook()
    partition_name = nc.partition_id_tensor.name if nc.partition_id_tensor else None
    in_names, out_names, out_avals = [], [], []
    for alloc in nc.m.functions[0].allocations:
        if not isinstance(alloc, mybir.MemoryLocationSet):
            continue
        name = alloc.memorylocations[0].name
        if alloc.kind == "ExternalInput":
            if name != partition_name:
                in_names.append(name)
        elif alloc.kind == "ExternalOutput":
            out_names.append(name)
            out_avals.append(
                jax.core.ShapedArray(
                    tuple(alloc.tensor_shape), mybir.dt.np(alloc.dtype)
                )
            )
    n_params = len(in_names)
    n_outs = len(out_names)
    all_names = in_names + out_names
    if partition_name is not None:
        all_names = all_names + [partition_name]

    def _body(*args):
        operands = list(args)
        if partition_name is not None:
            operands.append(bass2jax.partition_id_tensor())
        outs = bass2jax._bass_exec_p.bind(
            *operands,
            out_avals=tuple(out_avals),
            in_names=tuple(all_names),
            out_names=tuple(out_names),
            lowering_input_output_aliases=(),
            sim_require_finite=True,
            sim_require_nnan=True,
            nc=nc,
        )
        return tuple(outs)

    devices = jax.devices()[:N_CORES]
    mesh = Mesh(np.asarray(devices), ("core",))
    spec = NamedSharding(mesh, PartitionSpec("core"))
    sharded = jax.jit(
        shard_map(
            _body,
            mesh=mesh,
            in_specs=(PartitionSpec("core"),) * (n_params + n_outs),
            out_specs=(PartitionSpec("core"),) * n_outs,
            check_rep=False,
        ),
        donate_argnums=tuple(range(n_params, n_params + n_outs)),
        keep_unused=True,
    )
    zshapes = [(N_CORES * a.shape[0], *a.shape[1:]) for a in out_avals]
    zdtypes = [a.dtype for a in out_avals]
    zjit = jax.jit(
        lambda: tuple(jnp.zeros(s, d) for s, d in zip(zshapes, zdtypes)),
        out_shardings=(spec,) * n_outs,
    )
    # AOT-compile both executables now, single-threaded: concurrent first
    # calls from pipeline workers must not race into duplicate compiles.
    in_sds = []
    for alloc in nc.m.functions[0].allocations:
        if not isinstance(alloc, mybir.MemoryLocationSet):
            continue
        name = alloc.memorylocations[0].name
        if alloc.kind == "ExternalInput" and name != partition_name:
            shp = tuple(alloc.tensor_shape)
            in_sds.append(
                jax.ShapeDtypeStruct(
                    (N_CORES * shp[0], *shp[1:]), mybir.dt.np(alloc.dtype),
                    sharding=spec,
                )
            )
    z_sds = [
        jax.ShapeDtypeStruct(s, d, sharding=spec)
        for s, d in zip(zshapes, zdtypes)
    ]
    sharded_c = sharded.lower(*in_sds, *z_sds).compile()
    zjit_c = zjit.lower().compile()

    def put(np_global):
        import jax as _jax

        return _jax.device_put(np_global, spec)

    def put_shards(np_shards, dev_ids):
        """Async upload of per-core shard arrays to specific devices."""
        import jax as _jax

        return _jax.device_put(np_shards, [devices[c] for c in dev_ids])

    def assemble(shape, shard_arrays):
        import jax as _jax

        return _jax.make_array_from_single_device_arrays(
            shape, spec, shard_arrays
        )

    def call(*inputs):
        zs = zjit_c()
        outs = sharded_c(*inputs, *zs)
        return [np.asarray(o) for o in outs]

    def call_lazy(*inputs):
        """Dispatch and wait for execution, but leave results on device;
        the caller streams shards off (decode overlaps shard fetches)."""
        zs = zjit_c()
        outs = sharded_c(*inputs, *zs)
        jax.block_until_ready(outs)
        return outs

    return call, call_lazy, put, put_shards, assemble, in_names


_CACHE = {}


def _get_runner(n_img, A, B):
    key = (n_img, A, B)
    if key not in _CACHE:
        nc = _build_program(n_img, A, B)
        _CACHE[key] = _make_runner(nc)
    return _CACHE[key]


_PACK_M = np.uint64(0x8040201008040201)


def _host_l1_pack(x, w1, idx, out):
    """signbit(conv(x[idx], sign(w1))), bit-packed into out[k] as
    [16, 512, 64] u8 (channel-major; MSB-first; bit 1 = negative; rows
    510/511 zero).

    im2col uses full 512-wide rows so the junk columns 510/511 pad each
    row to exactly 64 bytes; sign bits are gathered 8-at-a-time with a
    uint64 multiply instead of np.packbits.  Channel-major lets gemm
    output blocks land without a transpose (the device unpack DMA does
    the partition interleave)."""
    W = np.sign(w1).astype(np.float32).reshape(16, 27)
    R = 30
    r_last = 510 - R
    nlast = R + 2                    # rows the last block reads (+2 dy halo)
    s56 = np.uint64(56)
    xe = np.zeros((3, nlast * 512 + 8), np.float32)
    col = np.empty((27, R * 512), np.float32)
    y = np.empty((16, R * 512), np.float32)
    for i, ix in enumerate(idx):
        xf = x[ix].reshape(3, -1)    # view; blocks before the last read it
        xe[:, : nlast * 512] = x[ix, :, r_last:, :].reshape(3, -1)
        o = out[i]
        o[:, 510:] = 0
        for r0 in range(0, 510, R):
            last = r0 == r_last
            src = xe if last else xf
            rb = 0 if last else r0
            j = 0
            for c in range(3):
                for dy in range(3):
                    base = (rb + dy) * 512
                    for dx in range(3):
                        col[j] = src[c, base + dx : base + dx + R * 512]
                        j += 1
            np.dot(W, col, out=y)
            sb = np.signbit(y)
            u = sb.view(np.uint8).view(np.uint64).reshape(16, -1)
            p8 = ((u * _PACK_M) >> s56).astype(np.uint8)
            o[:, r0 : r0 + R] = p8.reshape(16, R, 64)
    return out


# decode LUT indexed by the raw uint8 pattern of int8 byte
# p = s0 + 3*s1 + 9*s2 + 27*s3 + 81*s4 (balanced ternary digits in {-1,0,1})
_LUT = np.zeros((256, 5), np.float32)
for _s4 in (-1, 0, 1):
    for _s3 in (-1, 0, 1):
        for _s2 in (-1, 0, 1):
            for _s1 in (-1, 0, 1):
                for _s0 in (-1, 0, 1):
                    _p = _s0 + 3 * _s1 + 9 * _s2 + 27 * _s3 + 81 * _s4
                    _LUT[_p & 0xFF] = (_s0, _s1, _s2, _s3, _s4)

_WCACHE = {}


def _get_toeplitz(w2, w3):
    key = (w2.tobytes(), w3.tobytes())
    if key not in _WCACHE:
        _WCACHE[key] = _toeplitz_weights(w2, w3)
    return _WCACHE[key]


last_results = None
_EXEC = None
_JWCACHE = {}
_BUFS = {}


def _get_buf(r, shape):
    """Persistent per-round pk buffers (jax holds refs during the async
    upload, so each in-flight round needs its own)."""
    key = (r, shape)
    if key not in _BUFS:
        _BUFS[key] = np.empty(shape, np.uint8)
    return _BUFS[key]
# images per core per round: a small first round exposes less host-encode
# latency before the tunnel starts moving; later rounds' encode hides
# under earlier rounds' transfer + execution.
_CHUNK_PLAN = (1, 1, 1, 1)
_WORKERS = 4


def _decode_round(res_outp, out, idx, a3, b3):
    """res_outp: [n_round_imgs, 2, 510, npk] int8 (global, core-major)."""
    vals = _LUT[res_outp.view(np.uint8)]
    vals = vals.reshape(len(idx), 2, res_outp.shape[2], -1)
    for j, ix in enumerate(idx):
        out[ix] = vals[j, :, :a3, :b3]


def _get_jweights(nimg, A, B, t2, t3):
    """Device-resident replicated Toeplitz weights, cached across calls."""
    key = (nimg, A, B, t2.tobytes()[:64], t3.tobytes()[:64])
    if key not in _JWCACHE:
        call, call_lazy, put, put_shards, assemble, _ = _get_runner(nimg, A, B)
        jt2 = put(np.broadcast_to(t2, (N_CORES, *t2.shape)).reshape(
            N_CORES * t2.shape[0], t2.shape[1]).copy())
        jt3 = put(np.broadcast_to(t3, (N_CORES, *t3.shape)).reshape(
            N_CORES * t3.shape[0], t3.shape[1]).copy())
        _JWCACHE[key] = (call, call_lazy, put_shards, assemble, jt2, jt3)
    return _JWCACHE[key]


def kernel(inputs, w1, w2, w3):
    global last_results, _EXEC
    from concurrent.futures import ThreadPoolExecutor

    x = np.asarray(inputs, np.float32)
    w1 = np.asarray(w1, np.float32)
    n, _, A, B = x.shape
    per = n // N_CORES
    plan = _CHUNK_PLAN if sum(_CHUNK_PLAN) == per else (per,)
    t2, t3 = _get_toeplitz(
        np.asarray(w2, np.float32), np.asarray(w3, np.float32)
    )
    if _EXEC is None:
        _EXEC = ThreadPoolExecutor(_WORKERS)

    runners = {nimg: _get_jweights(nimg, A, B, t2, t3) for nimg in set(plan)}

    idxs, futs, start = [], [], 0
    for r, nimg in enumerate(plan):
        call, call_lazy, put_shards, assemble, jt2, jt3 = runners[nimg]
        idx = [per * i + start + j for i in range(N_CORES) for j in range(nimg)]
        # per-core encode with streamed uploads: each half-round's shards
        # start their (async) trip up the tunnel while the other half is
        # still encoding, so the final round's upload is mostly done by
        # the time its call is submitted.
        pk = _get_buf(r, (N_CORES, nimg, 16, 512, 64))
        shard_arrays = []
        for c0 in range(0, N_CORES, 4):
            for c in range(c0, c0 + 4):
                _host_l1_pack(
                    x, w1, idx[c * nimg : (c + 1) * nimg], pk[c]
                )
            shard_arrays.extend(
                put_shards(
                    [pk[c].reshape(-1) for c in range(c0, c0 + 4)],
                    list(range(c0, c0 + 4)),
                )
            )
        jpk = assemble((N_CORES * nimg * PIMG_BYTES,), shard_arrays)
        idxs.append(idx)
        last = r == len(plan) - 1
        futs.append(_EXEC.submit(call_lazy if last else call, jpk, jt2, jt3))
        start += nimg

    a3, b3 = A - 6, B - 6
    out = np.empty((n, 2, a3, b3), np.float32)
    last_results = None
    # decode rounds as they complete so the tail is one round's decode;
    # the final round streams shard-by-shard so decode overlaps fetches
    for c in range(len(plan)):
        outs = futs[c].result()
        if c == len(plan) - 1:
            nimg = plan[c]
            for shard in outs[0].addressable_shards:
                ci = shard.index[0].start // nimg
                po = np.asarray(shard.data)
                _decode_round(
                    po, out, idxs[c][ci * nimg : (ci + 1) * nimg], a3, b3,
                )
        else:
            _decode_round(
                outs[0].reshape(len(idxs[c]), 2, -1, outs[0].shape[-1]),
                out, idxs[c], a3, b3,
            )
    return out.reshape(n, -1)


# revision 32
# speedup vs baseline: 2.1277x; 2.1277x over previous
"""Trainium2 Bass kernel for a 3-layer binarized CNN (tunnel-optimized).

Network (reference):
    x  : [32, 3, 512, 512] fp32
    l1 : clip(conv(x, sign(w1)))            -> [32,16,510,510]
    l2 : clip(conv(sign(l1), sign(w2)))     -> [32,23,508,508]
    l3 : clip(conv(sign(l2), sign(w3)))     -> [32,2,506,506]
    out: l3.reshape(32, -1)

The axon tunnel moves ~75-140 MB/s up / ~25 MB/s down, charges ~0.08 s
per executable dispatch and ~0.1 s per output fetch, and the (single)
host CPU both encodes and feeds the tunnel, so end-to-end wall time is
dominated by host work, per-call latencies and bytes shipped -- not
device cycles.  Strategy:

  * Layer 1 only matters through the SIGN of its fp32 output.  It is
    computed on the host (one 16x27 sgemm per image, 512-wide junk-col
    im2col + uint64 sign-bit gather, channel-major so gemm blocks land
    without a transpose) and shipped as bit-packed signs: 17 MB instead
    of the 100 MB fp32 input.
  * The device unpacks bits to +-1 fp8 activations (DVE shift/and ops),
    then runs layers 2+3 exactly as integer-exact fp8 DoubleRow
    Toeplitz matmuls (data parallel over images).
  * The output (values in {-1,0,1}) is packed 5 columns/byte as
    balanced-ternary digits: p = s0+3*s1+9*s2+27*s3+81*s4 in int8,
    cutting the returned tensor 5x.  The host decodes with a 256x5 LUT.
  * One jax.jit executable is AOT-compiled once and reused every round;
    the Toeplitz weights ride up once as device-resident arrays and the
    donated output zero-buffers are created on-device, so per-round
    tunnel traffic is just packed signs up + ternary output down.
  * 4 rounds of 1 image/core pipeline through a 3-worker executor:
    per-core shards are device_put as soon as they are encoded (uploads
    stream under the remaining encode), later rounds' encode overlaps
    earlier rounds' execution, and round downloads ride the idle down-
    direction of the full-duplex tunnel.  The tail is one round's
    dispatch + exec + fetch + decode.
"""

import numpy as np

import concourse.bacc as bacc
import concourse.mybir as mybir
import concourse.tile as tile
from concourse import bass2jax

F32 = mybir.dt.float32
F16 = mybir.dt.float16
F8 = mybir.dt.float8e4
U8 = mybir.dt.uint8
I8 = mybir.dt.int8
ALU = mybir.AluOpType
DR = mybir.MatmulPerfMode.DoubleRow
SIGN = mybir.ActivationFunctionType.Sign

N_CORES = 8
AL2, AO2 = 7, 5      # L2: rows window / rows out per block
AL3, AO3 = 32, 30    # L3
C2, C3 = 16, 23
O2, O3 = 23, 2


def _toeplitz_weights(w2, w3):
    """Build the stationary Toeplitz matrices (host side)."""
    s2 = np.sign(w2).astype(np.float32)  # [23,16,3,3]
    s3 = np.sign(w3).astype(np.float32)  # [2,23,3,3]

    # T2[(al*8 + cp), dx, codd, (aol*23 + PI[o])] fp8 DoubleRow pairs, M pad
    # 128.  PI permutes the 23 output channels into the (c4, cc, t) order of
    # the s2 DRAM layout so layer 3 can load its rhs with a single DMA:
    # o = cc*8 + c4*2 + t  is stored at  c' = c4*6 + cc*2 + t.
    PI = [((o % 8) // 2) * 6 + (o // 8) * 2 + (o % 2) for o in range(O2)]
    t2 = np.zeros((56, 3, 2, 128), np.float32)
    for al in range(AL2):
        for aol in range(AO2):
            dy = al - aol
            if 0 <= dy <= 2:
                for c in range(C2):
                    for o in range(O2):
                        t2[al * 8 + c // 2, :, c % 2, aol * 23 + PI[o]] = s2[o, c, dy, :]
    # T3[(al*4 + cp), cc, dx, codd, (o*30 + aol)] fp8 DoubleRow, M pad 64.
    # M index is o-major so the output lands channel-major in DRAM.
    t3 = np.zeros((128, 3, 3, 2, 64), np.float32)
    for al in range(AL3):
        for aol in range(AO3):
            dy = al - aol
            if 0 <= dy <= 2:
                for cc in range(3):
                    for cl in range(8):
                        c = cc * 8 + cl
                        if c < C3:
                            for o in range(O3):
                                t3[al * 4 + cl // 2, cc, :, cl % 2, o * 30 + aol] = (
                                    s3[o, c, dy, :]
                                )
    import ml_dtypes

    return (
        t2.reshape(56, 3 * 256).astype(ml_dtypes.float8_e4m3).view(np.uint8),
        t3.reshape(128, 9 * 128).astype(ml_dtypes.float8_e4m3).view(np.uint8),
    )


PIMG_BYTES = 512 * 16 * 64         # packed l1 signs per image (512 rows, 2 pad)


def _build_program(n_img, A, B):
    """Emit the per-core SPMD Bass program (unpack + layers 2,3)."""
    n1, n2, n3 = B - 2, B - 4, B - 6          # 510, 508, 506
    nblk2 = -(-(A - 4) // AO2)                 # 102
    nblk3 = -(-(A - 6) // AO3)                 # 17
    a1 = AL2 + AO2 * (nblk2 - 1)               # 512 s1 rows (incl zero tail)
    s2a = max(AO2 * nblk2, AO3 * (nblk3 - 1) + AL3)  # 512
    npk = (n3 + 4) // 5                        # 102 packed output bytes (base-3)

    nc = bacc.Bacc("TRN2", target_bir_lowering=False, debug=False)

    # pk layout per image: [16 ch, 512 rows, 64 B] (channel-major: the host
    # packer writes gemm output blocks without a transpose; the unpack DMA
    # below does the (row%16, ch-pair) partition interleave instead).
    pk_t = nc.dram_tensor("pk", [n_img * PIMG_BYTES], U8, kind="ExternalInput")
    s1p = pk_t.ap().rearrange(
        "(i c2 t g r16 b) -> i r16 c2 g t b",
        i=n_img, c2=8, t=2, g=a1 // 16, r16=16, b=64,
    )
    t2t = nc.dram_tensor("t2w", [56, 3 * 256], U8, kind="ExternalInput")
    t3t = nc.dram_tensor("t3w", [128, 9 * 128], U8, kind="ExternalInput")
    t2w = t2t.ap().bitcast(F8)
    t3w = t3t.ap().bitcast(F8)
    outp = nc.dram_tensor(
        "outp", [n_img, 2, AO3 * nblk3, npk], I8, kind="ExternalOutput"
    )
    # one spare junk row (index 512): the last L2 slab's second half-load
    # reads 8 rows starting at 505; row 512 is loaded but never consumed.
    s1d = [
        nc.dram_tensor(f"s1_{i}", [a1 + 1, 16, n1], F8, kind="Internal")
        for i in range(n_img)
    ]
    s2d = [
        nc.dram_tensor(f"s2_{i}", [s2a, 24, n2], F8, kind="Internal")
        for i in range(n_img)
    ]

    ngrp = a1 // 16                            # 32 packed row-groups
    nslab = (AO2 * nblk2) // 10                # 51 L2 double-block slabs

    with tile.TileContext(nc) as tc:
        with (
            tc.tile_pool(name="const", bufs=1) as cpool,
            tc.tile_pool(name="unp", bufs=2) as pu,
            tc.tile_pool(name="l2", bufs=4) as p2,
            tc.tile_pool(name="l3", bufs=4) as p3,
            tc.tile_pool(name="ps2", bufs=2, space="PSUM") as ps2p,
            tc.tile_pool(name="ps3", bufs=2, space="PSUM") as ps3p,
        ):
            # t2 stationary duplicated at partition bases 0 and 64: matmul
            # requires lhsT.base_partition == rhs.base_partition, and the
            # second block of each L2 slab reads rhs partitions 64..119.
            t2sb = cpool.tile([120, 3 * 256], F8)
            t3sb = cpool.tile([128, 9 * 128], F8)
            ztile = cpool.tile([128, B], F8)
            nc.sync.dma_start(t2sb[0:56, :], t2w)
            nc.sync.dma_start(t2sb[64:120, :], t2w)
            nc.sync.dma_start(t3sb[:], t3w)
            nc.vector.memset(ztile[:], 0.0)

            for img in range(n_img):
                s1, s2 = s1d[img].ap(), s2d[img].ap()
                # ---- zero pads of s2: channel-23 plane + tail rows ----
                for r in range(0, s2a, 128):
                    cnt = min(128, s2a - r)
                    nc.sync.dma_start(s2[r : r + cnt, 23, :], ztile[:cnt, :n2])
                for a in range(AO2 * nblk2, s2a):
                    nc.sync.dma_start(s2[a, :, :], ztile[:24, :n2])

                # ------- unpack host L1 sign bits -> s1, whole image -------
                # partitions (row%16, ch-pair); free dims (group, bytes)
                pk = pu.tile([128, 128 * ngrp], U8, tag="pk")
                pkv = pk[:].rearrange(
                    "(r16 c2) (g t b) -> c2 t r16 g b", r16=16, g=ngrp, t=2
                )
                for c2 in range(8):
                    for t in range(2):
                        nc.sync.dma_start(
                            pkv[c2, t], s1p[img][:, c2, :, t, :]
                        )
                upk = pu.tile([128, 1024 * ngrp], F8, tag="upk")
                upkv = upk[:].rearrange("p (q f) -> p q f", f=8)
                for k in range(8):
                    tb = pu.tile([128, 128 * ngrp], U8, tag="tb")
                    nc.vector.tensor_scalar(
                        tb[:], pk[:], int(7 - k), int(1),
                        op0=ALU.logical_shift_right, op1=ALU.bitwise_and,
                    )
                    # host packs raw sign bits (1 = negative): sign = 1 - 2b
                    nc.vector.tensor_scalar(
                        upkv[:, :, k], tb[:], -2.0, 1.0,
                        op0=ALU.mult, op1=ALU.add,
                    )
                for tt in range(2):
                    nc.sync.dma_start(
                        s1[0:a1].rearrange(
                            "(g r) (c t) n -> (r c) t g n", g=ngrp, t=2
                        )[:, tt],
                        upk[:].rearrange("p (g t n) -> p t g n", g=ngrp, t=2)[
                            :, tt, :, 0:n1
                        ],
                    )

                # --- layer 2: 10-row slabs as two 8-row half-loads (bases
                # 0/64), 2 blocks x 3 dx, fp8 DR, 2 PSUM banks, 1 Sign ---
                for s in range(nslab):
                    r0 = 10 * s
                    slab = p2.tile([128, 2 * 512], F8, tag="slab")
                    sv = slab[:].rearrange("k (t h) -> k t h", t=2)
                    nc.sync.dma_start(sv[0:64, :, 0:n1], s1[r0 : r0 + 8, :, :])
                    nc.sync.dma_start(
                        sv[64:128, :, 0:n1], s1[r0 + 5 : r0 + 13, :, :]
                    )
                    ps = ps2p.tile([115, 1024], F32, tag="ps2")
                    psv = ps[:].rearrange("p (g n) -> p g n", g=2)
                    for g in range(2):
                        for dx in range(3):
                            nc.tensor.matmul(
                                psv[:, g, 0:n2],
                                t2sb[64 * g : 64 * g + 56, :].rearrange(
                                    "k (x t m) -> k x t m", x=3, t=2
                                )[:, dx, :, 0:115],
                                sv[64 * g : 64 * g + 56, :, dx : dx + n2],
                                start=(dx == 0),
                                stop=(dx == 2),
                                perf_mode=DR,
                            )
                    sg2 = p2.tile([115, 2 * n2], F8, tag="sg2")
                    sg2v = sg2[:].rearrange("p (g n) -> p g n", g=2)
                    nc.scalar.activation(sg2v[:, :, :], psv[:, :, 0:n2], SIGN)
                    for g in range(2):
                        nc.sync.dma_start(
                            s2[r0 + 5 * g : r0 + 5 * g + 5, 0:23, :],
                            sg2v[:, g, :],
                        )

                # -------- layer 3 (fp8 DoubleRow) + 2-bit output pack --------
                for bb in range(nblk3):
                    rt = p3.tile([128, 3 * 2 * 512], F8, tag="rhs3")
                    rtv = rt[:].rearrange("k (b t h) -> k b t h", b=3, t=2)
                    nc.sync.dma_start(
                        rtv[:, :, :, 0:n2],
                        s2[30 * bb : 30 * bb + 32].rearrange(
                            "r (A b t) n -> (r A) b t n", A=4, t=2
                        ),
                    )
                    ps = ps3p.tile([60, n3], F32, tag="ps3")
                    for cc in range(3):
                        for dx in range(3):
                            nc.tensor.matmul(
                                ps[:],
                                t3sb[
                                    :, 128 * (cc * 3 + dx) : 128 * (cc * 3 + dx) + 128
                                ].rearrange("k (t m) -> k t m", t=2)[:, :, 0:60],
                                rtv[:, cc, :, dx : dx + n3],
                                start=(cc == 0 and dx == 0),
                                stop=(cc == 2 and dx == 2),
                                perf_mode=DR,
                            )
                    sg = p3.tile([60, 5 * npk], F16, tag="sg")
                    nc.vector.memset(sg[:, n3 : 5 * npk], 0.0)
                    nc.scalar.activation(sg[:, 0:n3], ps[:], SIGN)
                    sgv = sg[:].rearrange("p (n f) -> p n f", f=5)
                    pa = p3.tile([60, npk], F16, tag="pa")
                    nc.vector.scalar_tensor_tensor(
                        pa[:], sgv[:, :, 1], 3.0, sgv[:, :, 0],
                        op0=ALU.mult, op1=ALU.add,
                    )
                    pb = p3.tile([60, npk], F16, tag="pb")
                    nc.vector.scalar_tensor_tensor(
                        pb[:], sgv[:, :, 2], 9.0, pa[:],
                        op0=ALU.mult, op1=ALU.add,
                    )
                    pc = p3.tile([60, npk], F16, tag="pc")
                    nc.vector.scalar_tensor_tensor(
                        pc[:], sgv[:, :, 3], 27.0, pb[:],
                        op0=ALU.mult, op1=ALU.add,
                    )
                    po = p3.tile([60, npk], I8, tag="po")
                    nc.vector.scalar_tensor_tensor(
                        po[:], sgv[:, :, 4], 81.0, pc[:],
                        op0=ALU.mult, op1=ALU.add,
                    )
                    nc.sync.dma_start(
                        outp.ap()[img, :, 30 * bb : 30 * bb + 30, :], po[:]
                    )

    nc.compile()
    return nc


def _make_runner(nc):
    """Trace one reusable jax.jit for the SPMD program.

    Inputs ride as globally-concatenated arrays sharded over the 8 cores.
    The donated output zero-buffers are produced on-device by a second
    tiny jit, so they never cross the tunnel.
    Returns (call, put) where call(pk_np_or_jax, *resident) -> [np outs]
    and put(np_global) uploads a resident (replicated-by-tiling) input.
    """
    import jax
    import jax.numpy as jnp
    from jax.sharding import Mesh, PartitionSpec, NamedSharding
    from jax.experimental.shard_map import shard_map

    bass2jax.install_neuronx_cc_hook()
    partition_name = nc.partition_id_tensor.name if nc.partition_id_tensor else None
    in_names, out_names, out_avals = [], [], []
    for alloc in nc.m.functions[0].allocations:
        if not isinstance(alloc, mybir.MemoryLocationSet):
            continue
        name = alloc.memorylocations[0].name
        if alloc.kind == "ExternalInput":
            if name != partition_name:
                in_names.append(name)
        elif alloc.kind == "ExternalOutput":
            out_names.append(name)
            out_avals.append(
                jax.core.ShapedArray(
                    tuple(alloc.tensor_shape), mybir.dt.np(alloc.dtype)
                )
            )
    n_params = len(in_names)
    n_outs = len(out_names)
    all_names = in_names + out_names
    if partition_name is not None:
        all_names = all_names + [partition_name]

    def _body(*args):
        operands = list(args)
        if partition_name is not None:
            operands.append(bass2jax.partition_id_tensor())
        outs = bass2jax._bass_exec_p.bind(
            *operands,
            out_avals=tuple(out_avals),
            in_names=tuple(all_names),
            out_names=tuple(out_names),
            lowering_input_output_aliases=(),
            sim_require_finite=True,
            sim_require_nnan=True,
            nc=nc,
        )
        return tuple(outs)

    devices = jax.devices()[:N_CORES]
    mesh = Mesh(np.asarray(devices), ("core",))
    spec = NamedSharding(mesh, PartitionSpec("core"))
    sharded = jax.jit(
        shard_map(
            _body,
            mesh=mesh,
            in_specs=(PartitionSpec("core"),) * (n_params + n_outs),
            out_specs=(PartitionSpec("core"),) * n_outs,
            check_rep=False,
        ),
        donate_argnums=tuple(range(n_params, n_params + n_outs)),
        keep_unused=True,
    )
    zshapes = [(N_CORES * a.shape[0], *a.shape[1:]) for a in out_avals]
    zdtypes = [a.dtype for a in out_avals]
    zjit = jax.jit(
        lambda: tuple(jnp.zeros(s, d) for s, d in zip(zshapes, zdtypes)),
        out_shardings=(spec,) * n_outs,
    )
    # AOT-compile both executables now, single-threaded: concurrent first
    # calls from pipeline workers must not race into duplicate compiles.
    in_sds = []
    for alloc in nc.m.functions[0].allocations:
        if not isinstance(alloc, mybir.MemoryLocationSet):
            continue
        name = alloc.memorylocations[0].name
        if alloc.kind == "ExternalInput" and name != partition_name:
            shp = tuple(alloc.tensor_shape)
            in_sds.append(
                jax.ShapeDtypeStruct(
                    (N_CORES * shp[0], *shp[1:]), mybir.dt.np(alloc.dtype),
                    sharding=spec,
                )
            )
    z_sds = [
        jax.ShapeDtypeStruct(s, d, sharding=spec)
        for s, d in zip(zshapes, zdtypes)
    ]
    sharded_c = sharded.lower(*in_sds, *z_sds).compile()
    zjit_c = zjit.lower().compile()

    def put(np_global):
        import jax as _jax

        return _jax.device_put(np_global, spec)

    def put_shards(np_shards, dev_ids):
        """Async upload of per-core shard arrays to specific devices."""
        import jax as _jax

        return _jax.device_put(np_shards, [devices[c] for c in dev_ids])

    def assemble(shape, shard_arrays):
        import jax as _jax

        return _jax.make_array_from_single_device_arrays(
            shape, spec, shard_arrays
        )

    def call(*inputs):
        zs = zjit_c()
        outs = sharded_c(*inputs, *zs)
        return [np.asarray(o) for o in outs]

    def call_lazy(*inputs):
        """Dispatch and wait for execution, but leave results on device;
        the caller streams shards off (decode overlaps shard fetches)."""
        zs = zjit_c()
        outs = sharded_c(*inputs, *zs)
        jax.block_until_ready(outs)
        return outs

    return call, call_lazy, put, put_shards, assemble, in_names


_CACHE = {}


def _get_runner(n_img, A, B):
    key = (n_img, A, B)
    if key not in _CACHE:
        nc = _build_program(n_img, A, B)
        _CACHE[key] = _make_runner(nc)
    return _CACHE[key]


_PACK_M = np.uint64(0x8040201008040201)


def _host_l1_pack(x, w1, idx, out):
    """signbit(conv(x[idx], sign(w1))), bit-packed into out[k] as
    [16, 512, 64] u8 (channel-major; MSB-first; bit 1 = negative; rows
    510/511 zero).

    im2col uses full 512-wide rows so the junk columns 510/511 pad each
    row to exactly 64 bytes; sign bits are gathered 8-at-a-time with a
    uint64 multiply instead of np.packbits.  Channel-major lets gemm
    output blocks land without a transpose (the device unpack DMA does
    the partition interleave)."""
    W = np.sign(w1).astype(np.float32).reshape(16, 27)
    R = 30
    r_last = 510 - R
    nlast = R + 2                    # rows the last block reads (+2 dy halo)
    s56 = np.uint64(56)
    xe = np.zeros((3, nlast * 512 + 8), np.float32)
    col = np.empty((27, R * 512), np.float32)
    y = np.empty((16, R * 512), np.float32)
    for i, ix in enumerate(idx):
        xf = x[ix].reshape(3, -1)    # view; blocks before the last read it
        xe[:, : nlast * 512] = x[ix, :, r_last:, :].reshape(3, -1)
        o = out[i]
        o[:, 510:] = 0
        for r0 in range(0, 510, R):
            last = r0 == r_last
            src = xe if last else xf
            rb = 0 if last else r0
            j = 0
            for c in range(3):
                for dy in range(3):
                    base = (rb + dy) * 512
                    for dx in range(3):
                        col[j] = src[c, base + dx : base + dx + R * 512]
                        j += 1
            np.dot(W, col, out=y)
            sb = np.signbit(y)
            u = sb.view(np.uint8).view(np.uint64).reshape(16, -1)
            p8 = ((u * _PACK_M) >> s56).astype(np.uint8)
            o[:, r0 : r0 + R] = p8.reshape(16, R, 64)
    return out


# decode LUT indexed by the raw uint8 pattern of int8 byte
# p = s0 + 3*s1 + 9*s2 + 27*s3 + 81*s4 (balanced ternary digits in {-1,0,1})
_LUT = np.zeros((256, 5), np.float32)
for _s4 in (-1, 0, 1):
    for _s3 in (-1, 0, 1):
        for _s2 in (-1, 0, 1):
            for _s1 in (-1, 0, 1):
                for _s0 in (-1, 0, 1):
                    _p = _s0 + 3 * _s1 + 9 * _s2 + 27 * _s3 + 81 * _s4
                    _LUT[_p & 0xFF] = (_s0, _s1, _s2, _s3, _s4)

_WCACHE = {}


def _get_toeplitz(w2, w3):
    key = (w2.tobytes(), w3.tobytes())
    if key not in _WCACHE:
        _WCACHE[key] = _toeplitz_weights(w2, w3)
    return _WCACHE[key]


last_results = None
_EXEC = None
_JWCACHE = {}
_BUFS = {}


def _get_buf(r, shape):
    """Persistent per-round pk buffers (jax holds refs during the async
    upload, so each in-flight round needs its own)."""
    key = (r, shape)
    if key not in _BUFS:
        _BUFS[key] = np.empty(shape, np.uint8)
    return _BUFS[key]
# images per core per round: a small first round exposes less host-encode
# latency before the tunnel starts moving; later rounds' encode hides
# under earlier rounds' transfer + execution.
_CHUNK_PLAN = (1, 1, 1, 1)
_WORKERS = 3


def _decode_round(res_outp, out, idx, a3, b3):
    """res_outp: [n_round_imgs, 2, 510, npk] int8 (global, core-major)."""
    vals = _LUT[res_outp.view(np.uint8)]
    vals = vals.reshape(len(idx), 2, res_outp.shape[2], -1)
    for j, ix in enumerate(idx):
        out[ix] = vals[j, :, :a3, :b3]


def _get_jweights(nimg, A, B, t2, t3):
    """Device-resident replicated Toeplitz weights, cached across calls."""
    key = (nimg, A, B, t2.tobytes()[:64], t3.tobytes()[:64])
    if key not in _JWCACHE:
        call, call_lazy, put, put_shards, assemble, _ = _get_runner(nimg, A, B)
        jt2 = put(np.broadcast_to(t2, (N_CORES, *t2.shape)).reshape(
            N_CORES * t2.shape[0], t2.shape[1]).copy())
        jt3 = put(np.broadcast_to(t3, (N_CORES, *t3.shape)).reshape(
            N_CORES * t3.shape[0], t3.shape[1]).copy())
        _JWCACHE[key] = (call, call_lazy, put_shards, assemble, jt2, jt3)
    return _JWCACHE[key]


def kernel(inputs, w1, w2, w3):
    global last_results, _EXEC
    from concurrent.futures import ThreadPoolExecutor

    x = np.asarray(inputs, np.float32)
    w1 = np.asarray(w1, np.float32)
    n, _, A, B = x.shape
    per = n // N_CORES
    plan = _CHUNK_PLAN if sum(_CHUNK_PLAN) == per else (per,)
    t2, t3 = _get_toeplitz(
        np.asarray(w2, np.float32), np.asarray(w3, np.float32)
    )
    if _EXEC is None:
        _EXEC = ThreadPoolExecutor(_WORKERS)

    runners = {nimg: _get_jweights(nimg, A, B, t2, t3) for nimg in set(plan)}

    idxs, futs, start = [], [], 0
    for r, nimg in enumerate(plan):
        call, call_lazy, put_shards, assemble, jt2, jt3 = runners[nimg]
        idx = [per * i + start + j for i in range(N_CORES) for j in range(nimg)]
        # per-core encode with streamed uploads: each half-round's shards
        # start their (async) trip up the tunnel while the other half is
        # still encoding, so the final round's upload is mostly done by
        # the time its call is submitted.
        pk = _get_buf(r, (N_CORES, nimg, 16, 512, 64))
        shard_arrays = []
        for c0 in range(0, N_CORES, 4):
            for c in range(c0, c0 + 4):
                _host_l1_pack(
                    x, w1, idx[c * nimg : (c + 1) * nimg], pk[c]
                )
            shard_arrays.extend(
                put_shards(
                    [pk[c].reshape(-1) for c in range(c0, c0 + 4)],
                    list(range(c0, c0 + 4)),
                )
            )
        jpk = assemble((N_CORES * nimg * PIMG_BYTES,), shard_arrays)
        idxs.append(idx)
        futs.append(_EXEC.submit(call, jpk, jt2, jt3))
        start += nimg

    a3, b3 = A - 6, B - 6
    out = np.empty((n, 2, a3, b3), np.float32)
    last_results = None
    # decode rounds as they complete so the tail is one round's decode;
    # the final round streams shard-by-shard so decode overlaps fetches
    for c in range(len(plan)):
        outs = futs[c].result()
        _decode_round(
            outs[0].reshape(len(idxs[c]), 2, -1, outs[0].shape[-1]),
            out, idxs[c], a3, b3,
        )
    return out.reshape(n, -1)
